# revision 1
# baseline (speedup 1.0000x reference)
"""DGCNN Trainium2 Bass kernel.

Data-parallel over batch: 8 samples -> 8 NeuronCores, one sample per core.

Per-layer algorithm (exact reformulation of the reference):
  s[n,m] = <x_n,x_m> - |x_m|^2/2      (row-order-equivalent to reference pd)
  knn(n) = top-10 columns of s row n  (exact fp32 top-k, max8 x2 rounds)
  u = Wa @ x, z = (Wb - Wa) @ x       where W = [Wa | Wb]
  x_next[:, n] = lrelu(z[:, n] + max_{m in knn(n)} u[:, m])
    (lrelu monotone => max-then-lrelu == reference lrelu-then-max)

s is computed per 512-col block as a K=C matmul plus a K=1 bias matmul
(lhsT = ones row, rhs = -xx/2) accumulating into the same PSUM region.
Top-k runs on DVE straight from PSUM. u rows are gathered from an internal
DRAM buffer by 10 per-neighbor indirect DMAs, then reduced with a strided
max. PE transposes bring x_next back to channel-major.

All DMA-loaded weights are laundered through one ACT copy so downstream
consumers depend on a single engine (ISA sync-wait slots are scarce).
"""

import numpy as np

import concourse.bass as bass
import concourse.mybir as mybir
import concourse.tile as tile
from concourse.tile_rust import add_dep_helper
from concourse.bass_utils import run_bass_kernel_spmd
from concourse.masks import make_identity

P = 128
N = 2048
NT = N // P              # 16 row tiles
KNN = 10
NEG = -1e30

# (C_in, O) per edge layer
LAYERS = [(3, 64), (64, 64), (64, 128), (128, 256)]

F32 = mybir.dt.float32
U32 = mybir.dt.uint32


def _lrelu(nc, pool, out_sb, t_ap, tag, shape):
    """out_sb = lrelu(t_ap) = t + 0.8*relu(-t). t_ap may be PSUM or SBUF."""
    rr = pool.tile(shape, F32, tag=f"rr_{tag}")
    nc.scalar.activation(out=rr, in_=t_ap,
                         func=mybir.ActivationFunctionType.Relu, scale=-1.0)
    nc.vector.scalar_tensor_tensor(
        out=out_sb, in0=rr, scalar=0.8, in1=t_ap,
        op0=mybir.AluOpType.mult, op1=mybir.AluOpType.add)


def _pe_join(nc, ident, pj, tgt_cell, dep_cells, after_pe=(), after_other=(),
             skip=True):
    """Dummy 1x1 matmuls so the real matmuls that follow carry <=1 sync wait
    (the ISA matmul has one wait slot). Dummies into the pejoin scratch make
    PE observe: all prior-PE instructions (one dummy, PE sems merge), each
    non-PE instruction (one dummy apiece), and each dependency tensor's
    producer. A final dummy into the target PSUM cell absorbs the slot's
    recycling wait."""
    ic = ident[0:1, 0:1]
    pjc = pj[0:1, 0:1]
    if after_pe:
        d0 = nc.tensor.matmul(pjc, ic, ic, start=True, stop=True,
                              skip_group_check=skip)
        for a in after_pe:
            add_dep_helper(d0.ins, a.ins, sync=True,
                           reason="pe_join absorbs PE wait")
    for a in after_other:
        d = nc.tensor.matmul(pjc, ic, ic, start=True, stop=True,
                             skip_group_check=skip)
        add_dep_helper(d.ins, a.ins, sync=True,
                       reason="pe_join absorbs cross-engine wait")
    ds = []
    for c in dep_cells:
        ds.append(nc.tensor.matmul(pjc, c, c, start=True, stop=True,
                                   skip_group_check=skip))
    if tgt_cell is not None:
        nc.tensor.matmul(tgt_cell, ic, ic, start=True, stop=True,
                         skip_group_check=skip)
    return ds


def build_program(num_devices=8, debug_taps=False):
    nc = bass.Bass("TRN2", target_bir_lowering=False, debug=False,
                   enable_asserts=False, num_devices=num_devices)

    # ---- I/O ----
    x_in = nc.dram_tensor("x", (3, N), F32, kind="ExternalInput")
    w_in = {}
    for li, (c, o) in enumerate(LAYERS, start=1):
        w_in[f"waT{li}"] = nc.dram_tensor(f"waT{li}", (c, o), F32, kind="ExternalInput")
        w_in[f"wzT{li}"] = nc.dram_tensor(f"wzT{li}", (c, o), F32, kind="ExternalInput")
    w5T_in = nc.dram_tensor("w5T", (512, 512), F32, kind="ExternalInput")
    l1T_in = nc.dram_tensor("l1T", (1024, 512), F32, kind="ExternalInput")
    l2T_in = nc.dram_tensor("l2T", (512, 256), F32, kind="ExternalInput")
    b2_in = nc.dram_tensor("b2", (256,), F32, kind="ExternalInput")
    out_dram = nc.dram_tensor("out", (256,), F32, kind="ExternalOutput")
    taps = {}
    if debug_taps:
        for li, (c, o) in enumerate(LAYERS, start=1):
            taps[li] = nc.dram_tensor(f"xtap{li}", (o, N), F32, kind="ExternalOutput")

    # internal DRAM u buffers, token-major [N, O]
    u_dram = [nc.dram_tensor(f"u{li}", (N, o), F32, kind="Internal")
              for li, (c, o) in enumerate(LAYERS, start=1)]

    with tile.TileContext(nc) as tc:
        with (
            tc.tile_pool(name="mp", bufs=1) as mp,       # persistent / per-layer
            tc.tile_pool(name="rot", bufs=2) as rot,     # per-tile rotating
            tc.tile_pool(name="ps", bufs=1, space="PSUM") as ps,
            tc.tile_pool(name="ps2", bufs=2, space="PSUM") as ps2,
        ):
            # ---- constants ----
            ident = mp.tile([P, P], F32, tag="ident")
            make_identity(nc, ident)
            halfneg = mp.tile([P, 1], F32, tag="halfneg")
            nc.vector.memset(halfneg, -0.5)
            ones_row = mp.tile([1, P], F32, tag="ones_row")
            nc.vector.memset(ones_row, 1.0)
            pscrap = mp.tile([1, NT], F32, tag="pscrap")
            # dummy transpose so PE observes Pool's ident write early
            pj = ps2.tile([P, P], F32, tag="pejoin", bufs=1)
            nc.tensor.transpose(out=pj, in_=ident, identity=ident)

            # ---- load + launder input x ----
            x0 = mp.tile([3, N], F32, tag="x0")
            nc.sync.dma_start(out=x0, in_=x_in.ap())

            feats = {0: [x0]}
            x1 = mp.tile([64, N], F32, tag="x1")
            x2 = mp.tile([64, N], F32, tag="x2")
            x3 = mp.tile([128, N], F32, tag="x3")
            x4a = mp.tile([128, N], F32, tag="x4a")
            x4b = mp.tile([128, N], F32, tag="x4b")
            outs_ch = {1: [x1], 2: [x2], 3: [x3], 4: [x4a, x4b]}

            last_tps = []      # rolling window of recent transpose insts
            prev_mms = []      # all s-matmuls of the previous tile
            prev_dve = []      # all DVE readers of the previous s_ps tile
            prev_mm8 = None
            prev_mr = None
            for li, (C, O) in enumerate(LAYERS, start=1):
                x_ch = feats[li - 1][0]   # [C, N] channel-major input
                ud = u_dram[li - 1]

                # -- weights: DMA then ACT launder --
                waT_r = mp.tile([C, O], F32, tag=f"waT_r{li}")
                wzT_r = mp.tile([C, O], F32, tag=f"wzT_r{li}")
                nc.sync.dma_start(out=waT_r, in_=w_in[f"waT{li}"].ap())
                nc.sync.dma_start(out=wzT_r, in_=w_in[f"wzT{li}"].ap())
                waT = mp.tile([C, O], F32, tag="waT")
                wzT = mp.tile([C, O], F32, tag="wzT")
                nc.scalar.copy(out=waT, in_=waT_r)
                nc.scalar.copy(out=wzT, in_=wzT_r)

                # -- -xx/2 row --
                xsq = mp.tile([C, N], F32, tag="xsq")
                nc.vector.tensor_tensor(out=xsq, in0=x_ch, in1=x_ch,
                                        op=mybir.AluOpType.mult)
                negxx = mp.tile([1, N], F32, tag="negxx")
                for cblk in range(4):
                    sl = slice(cblk * 512, (cblk + 1) * 512)
                    nx_ps = ps2.tile([1, 512], F32, tag="sm_ps")
                    if cblk == 0:
                        ljd = _pe_join(nc, ident, pj, None,
                                       [xsq[0:1, 0:1], x_ch[0:1, 0:1],
                                        halfneg[0:1, 0:1], ones_row[0:1, 0:1]])
                    nc.scalar.copy(out=nx_ps[0:1, 0:1], in_=ident[0:1, 0:1])
                    mmx = nc.tensor.matmul(nx_ps, halfneg[0:C, :], xsq[:, sl],
                                           start=True, stop=True)
                    for d in ljd:
                        add_dep_helper(mmx.ins, d.ins, sync=False,
                                       reason="keep join dummies first")
                    nc.scalar.copy(out=negxx[:, sl], in_=nx_ps)

                # -- u^T (staged, single DRAM store), z^T --
                zT_all = mp.tile([P, NT * O], F32, tag="zT_all")
                uT_all = mp.tile([P, NT * O], F32, tag="uT_all")
                for t in range(NT):
                    tsl = slice(t * P, (t + 1) * P)
                    uz_ps = ps2.tile([P, O], F32, tag="sm_ps")
                    nc.scalar.copy(out=uz_ps[0:1, 0:1], in_=ident[0:1, 0:1])
                    nc.tensor.matmul(uz_ps, x_ch[:, tsl], waT, start=True, stop=True)
                    nc.scalar.copy(out=uT_all[:, t * O:(t + 1) * O], in_=uz_ps)
                    uz_ps2 = ps2.tile([P, O], F32, tag="sm_ps")
                    nc.scalar.copy(out=uz_ps2[0:1, 0:1], in_=ident[0:1, 0:1])
                    nc.tensor.matmul(uz_ps2, x_ch[:, tsl], wzT, start=True, stop=True)
                    nc.scalar.copy(out=zT_all[:, t * O:(t + 1) * O], in_=uz_ps2)
                nc.gpsimd.tensor_copy(
                    pscrap, uT_all.rearrange("p (t o) -> p t o", t=NT)[0:1, :, 3])
                ust = nc.gpsimd.dma_start(
                    out=ud.ap().rearrange("(t p) o -> p t o", p=P),
                    in_=uT_all)
                # Pool and ACT observe the store completion (WAR on the DMA's
                # read) so neither the per-tile gathers nor the next layer's
                # uT_all copies need a DMAHW wait slot of their own.
                nc.gpsimd.memset(uT_all[0:1, 0:1], 0.0)
                nc.scalar.mul(uT_all[0:1, 1:2], uT_all[0:1, 1:2], 0.0)

                # -- per row-tile pipeline --
                for t in range(NT):
                    tsl = slice(t * P, (t + 1) * P)
                    s_ps = ps.tile([P, N], F32, tag="s_ps")
                    # first toucher of the recycled slot: DVE memset -- its WAR
                    # (DVE readers) self-elides, so it carries only the PE WAW;
                    # the matmuls then wait on DVE alone.
                    nc.vector.memset(s_ps[0:1, 0:1], 0.0)
                    cur_mms = []
                    for cblk in range(4):
                        sl = slice(cblk * 512, (cblk + 1) * 512)
                        smm = nc.tensor.matmul(
                            s_ps[:, sl], x_ch[:, tsl], x_ch[:, sl],
                            start=True, stop=False)
                        if t == 0 and cblk == 0:
                            for d in ljd:
                                add_dep_helper(smm.ins, d.ins, sync=False,
                                               reason="keep join dummies first")
                        cur_mms.append(smm)
                        cur_mms.append(nc.tensor.matmul(
                            s_ps[:, sl], ones_row, negxx[:, sl],
                            start=False, stop=True))
                    prev_mms = cur_mms
                    prev_mm8 = cur_mms[-1]

                    v1 = rot.tile([P, 8], F32, tag="v1")
                    i1 = rot.tile([P, 8], U32, tag="i1")
                    v2 = rot.tile([P, 8], F32, tag="v2")
                    i2 = rot.tile([P, 8], U32, tag="i2")
                    s2 = rot.tile([P, N], F32, tag="s2", bufs=1)
                    mx1 = nc.vector.max(out=v1, in_=s_ps)
                    mi1 = nc.vector.max_index(out=i1, in_max=v1, in_values=s_ps)
                    mri = nc.vector.match_replace(out=s2, in_to_replace=v1,
                                                  in_values=s_ps,
                                                  imm_value=NEG)
                    prev_dve = [mx1, mi1, mri]
                    prev_mr = mri
                    nc.vector.max(out=v2, in_=s2)
                    nc.vector.max_index(out=i2, in_max=v2, in_values=s2)
                    idx10 = rot.tile([P, KNN], U32, tag="idx10")
                    nc.gpsimd.tensor_copy(idx10[:, 0:8], i1)
                    nc.gpsimd.tensor_copy(idx10[:, 8:KNN], i2[:, 0:2])

                    gth = rot.tile([P, KNN * O], F32, tag=f"gth{li}")
                    for j in range(KNN):
                        nc.gpsimd.indirect_dma_start(
                            out=gth[:, j * O:(j + 1) * O],
                            out_offset=None,
                            in_=ud.ap(),
                            in_offset=bass.IndirectOffsetOnAxis(
                                ap=idx10[:, j:j + 1], axis=0),
                        )

                    # chained max over the 10 gathered neighbor blocks; one
                    # DMA-lane wait per op (the ISA wait slot is single)
                    M = rot.tile([P, O], F32, tag="M")
                    nc.vector.tensor_copy(M, gth[:, 0:O])
                    for j in range(1, KNN):
                        nc.vector.tensor_tensor(
                            out=M, in0=M, in1=gth[:, j * O:(j + 1) * O],
                            op=mybir.AluOpType.max)

                    new_tps = []
                    tadd = rot.tile([P, O], F32, tag="tadd")
                    nc.vector.tensor_tensor(out=tadd, in0=M,
                                            in1=zT_all[:, t * O:(t + 1) * O],
                                            op=mybir.AluOpType.add)
                    xnT = rot.tile([P, O], F32, tag="xnT")
                    _lrelu(nc, rot, xnT, tadd, "xn", [P, O])

                    for ob in range((O + P - 1) // P):
                        cols = min(P, O - ob * P)
                        tp_ps = ps2.tile([cols, P], F32, tag="sm_ps")
                        nc.scalar.copy(out=tp_ps[0:1, 0:1], in_=ident[0:1, 0:1])
                        _pe_join(nc, ident, pj, None, [xnT[0:1, 0:1]])
                        tpi = nc.tensor.transpose(
                            out=tp_ps, in_=xnT[:, ob * P:ob * P + cols],
                            identity=ident)
                        new_tps.append(tpi)
                        nc.scalar.copy(out=outs_ch[li][ob][:cols, tsl], in_=tp_ps)
                    last_tps = (last_tps + new_tps)[-4:]

                feats[li] = outs_ch[li]
                # (last tile's transposes feed the next join)
                if debug_taps:
                    for ob, xt in enumerate(outs_ch[li]):
                        nc.gpsimd.tensor_copy(
                            pscrap,
                            xt.rearrange("c (t q) -> c t q", t=NT)[0:1, :, 0])
                        nc.gpsimd.dma_start(
                            out=taps[li].ap()[ob * P:ob * P + xt.shape[0], :],
                            in_=xt)

            # ---- W5 conv + lrelu + pooling ----
            kchunks = [(x1, 64, 0), (x2, 64, 64), (x3, 128, 128),
                       (x4a, 128, 256), (x4b, 128, 384)]
            w5sb = []
            for (src, kc, row0) in kchunks:
                wr = mp.tile([kc, 512], F32, tag=f"w5raw{row0}")
                nc.sync.dma_start(out=wr, in_=w5T_in.ap()[row0:row0 + kc, :])
                wt = mp.tile([kc, 512], F32, tag=f"w5_{row0}")
                nc.scalar.copy(out=wt, in_=wr)
                w5sb.append(wt)
            fmax = mp.tile([P, 4], F32, tag="fmax")
            fsum = mp.tile([P, 4], F32, tag="fsum")
            for m in range(4):
                msl = slice(m * P, (m + 1) * P)
                h_ps = ps.tile([P, N], F32, tag="s_ps")
                hm = nc.vector.memset(h_ps[0:1, 0:1], 0.0)
                _pe_join(nc, ident, pj,
                         None, [w[0:1, 0:1] for w in w5sb] if m == 0 else [],
                         after_other=[hm])
                for cblk in range(4):
                    sl = slice(cblk * 512, (cblk + 1) * 512)
                    for ci, (src, kc, row0) in enumerate(kchunks):
                        mmh = nc.tensor.matmul(h_ps[:, sl], w5sb[ci][:, msl],
                                               src[:, sl], start=(ci == 0),
                                               stop=(ci == len(kchunks) - 1))
                prev_mms = [mmh]
                last_tps = []
                h_sb = rot.tile([P, N], F32, tag="s2", bufs=1)
                t1 = mp.tile([P, N], F32, tag="xsq")
                tmin = nc.vector.tensor_scalar_min(t1, h_ps, 0.0)
                tstt = nc.vector.scalar_tensor_tensor(
                    out=h_sb, in0=t1, scalar=-0.8, in1=h_ps,
                    op0=mybir.AluOpType.mult, op1=mybir.AluOpType.add)
                prev_dve = [tmin, tstt]
                nc.vector.tensor_reduce(out=fmax[:, m:m + 1], in_=h_sb,
                                        axis=mybir.AxisListType.X,
                                        op=mybir.AluOpType.max)
                nc.vector.tensor_reduce(out=fsum[:, m:m + 1], in_=h_sb,
                                        axis=mybir.AxisListType.X,
                                        op=mybir.AluOpType.add)

            # ---- FC1 (mean-pool divisor folded into l1T rows 512:) ----
            l1sb = []
            for k in range(8):
                wt = mp.tile([P, 512], F32, tag=f"l1_{k}")
                nc.sync.dma_start(out=wt, in_=l1T_in.ap()[k * P:(k + 1) * P, :])
                l1sb.append(wt)
            fvec = mp.tile([P, 4], F32, tag="fvec")
            for m in range(4):
                msl = slice(m * P, (m + 1) * P)
                f_ps = ps2.tile([P, 1], F32, tag="fc_ps", bufs=1)
                nc.vector.memset(f_ps[0:1, 0:1], 0.0)
                if m == 0:
                    fjd = _pe_join(nc, ident, pj, None,
                                   [w[0:1, 0:1] for w in l1sb]
                                   + [fmax[0:1, 0:1], fsum[0:1, 0:1]])
                cur_mms = []
                for k in range(8):
                    rhs = fmax[:, k:k + 1] if k < 4 else fsum[:, k - 4:k - 3]
                    mmf = nc.tensor.matmul(f_ps, l1sb[k][:, msl], rhs,
                                           start=(k == 0), stop=(k == 7))
                    if k == 0:
                        for d in fjd:
                            add_dep_helper(mmf.ins, d.ins, sync=False,
                                           reason="join dummies first")
                    cur_mms.append(mmf)
                prev_mms = cur_mms
                ft = mp.tile([P, 1], F32, tag=f"ft{m}")
                a1 = nc.vector.tensor_scalar_min(ft, f_ps, 0.0)
                a2 = nc.vector.scalar_tensor_tensor(
                    out=fvec[:, m:m + 1], in0=ft, scalar=-0.8, in1=f_ps,
                    op0=mybir.AluOpType.mult, op1=mybir.AluOpType.add)
                prev_dve = [a1, a2]

            # ---- FC2 + b2 + lrelu -> out ----
            l2sb = []
            for k in range(4):
                wt = mp.tile([P, 256], F32, tag=f"l2_{k}")
                nc.sync.dma_start(out=wt, in_=l2T_in.ap()[k * P:(k + 1) * P, :])
                l2sb.append(wt)
            b2r = mp.tile([P, 2], F32, tag="b2r")
            nc.sync.dma_start(out=b2r,
                              in_=b2_in.ap().rearrange("(m p) -> p m", m=2))
            b2sb = mp.tile([P, 2], F32, tag="b2sb")
            nc.vector.tensor_copy(b2sb, b2r)
            for m in range(2):
                msl = slice(m * P, (m + 1) * P)
                o_ps = ps2.tile([P, 1], F32, tag="fc_ps", bufs=1)
                nc.vector.memset(o_ps[0:1, 0:1], 0.0)
                if m == 0:
                    ojd = _pe_join(nc, ident, pj, None,
                                   [w[0:1, 0:1] for w in l2sb] + [fvec[0:1, 0:1]])
                cur_mms = []
                for k in range(4):
                    mmo = nc.tensor.matmul(o_ps, l2sb[k][:, msl],
                                           fvec[:, k:k + 1],
                                           start=(k == 0), stop=(k == 3))
                    if k == 0:
                        for d in ojd:
                            add_dep_helper(mmo.ins, d.ins, sync=False,
                                           reason="join dummies first")
                    cur_mms.append(mmo)
                prev_mms = cur_mms
                ob = mp.tile([P, 1], F32, tag=f"ob{m}")
                nc.vector.tensor_tensor(out=ob, in0=o_ps, in1=b2sb[:, m:m + 1],
                                        op=mybir.AluOpType.add)
                ofin = mp.tile([P, 1], F32, tag=f"ofin{m}")
                ot = mp.tile([P, 1], F32, tag=f"ot{m}")
                b1 = nc.vector.tensor_scalar_min(ot, ob, 0.0)
                badd = nc.vector.tensor_tensor
                b2i = nc.vector.scalar_tensor_tensor(
                    out=ofin, in0=ot, scalar=-0.8, in1=ob,
                    op0=mybir.AluOpType.mult, op1=mybir.AluOpType.add)
                prev_dve = [b1, b2i]
                nc.gpsimd.tensor_copy(pscrap[0:1, 0:1], ofin[0:1, 0:1])
                lastdma = nc.gpsimd.dma_start(
                    out=out_dram.ap()[m * P:(m + 1) * P], in_=ofin)
            prevn = lastdma
            for _ in range(24):
                nn_ = nc.sync.nop()
                add_dep_helper(nn_.ins, prevn.ins, sync=False,
                               reason="keep drain-slack nops last")
                prevn = nn_

    import os
    if not os.environ.get('KERNEL_NO_FIXUP'):
        _fix_waits(nc)
    return nc


def _fix_waits(nc):
    """Post-schedule wait-slot legalization. Each engine instruction may carry
    only ONE sync wait on TRN2. Split any excess waits onto NoOp instructions
    inserted immediately before the over-cap instruction on the same engine
    queue -- all waits still execute before the instruction dispatches, so
    ordering semantics are unchanged."""
    import concourse.mybir as mybir
    ctr = [0]
    for f in nc.m.functions:
        for bb in f.blocks:
            il = list(bb.instructions)
            out = []
            changed = False
            for i in il:
                si = i.sync_info
                n = len(si.on_wait) if (si and si.on_wait) else 0
                if n > 1:
                    w = list(si.on_wait)
                    for extra in w[:-1]:
                        ctr[0] += 1
                        nop = mybir.InstNoOp(name=f"waitnop-{ctr[0]}",
                                             ins=[], outs=[])
                        nop.engine = i.engine
                        nop.sync_info = mybir.SyncInfo(on_wait=[extra],
                                                       on_update=[])
                        out.append(nop)
                    i.sync_info = mybir.SyncInfo(on_wait=[w[-1]],
                                                 on_update=si.on_update)
                    changed = True
                out.append(i)
            if changed:
                bb.instructions = out


def _host_weights(W1, W2, W3, W4, W5, L1, L2, b2):
    ws = {}
    for li, (C, O), W in zip(range(1, 5), LAYERS, [W1, W2, W3, W4]):
        Wa = W[:, :C]
        Wz = W[:, C:] - Wa
        ws[f"waT{li}"] = np.ascontiguousarray(Wa.T.astype(np.float32))
        ws[f"wzT{li}"] = np.ascontiguousarray(Wz.T.astype(np.float32))
    ws["w5T"] = np.ascontiguousarray(W5.T.astype(np.float32))
    l1T = L1.T.astype(np.float32).copy()
    l1T[512:, :] *= np.float32(1.0 / N)   # fold mean-pool divisor
    ws["l1T"] = np.ascontiguousarray(l1T)
    ws["l2T"] = np.ascontiguousarray(L2.T.astype(np.float32))
    ws["b2"] = np.ascontiguousarray(b2.astype(np.float32))
    return ws


_prog_cache = {}


def _get_prog(debug_taps=False):
    key = debug_taps
    if key not in _prog_cache:
        _prog_cache[key] = build_program(num_devices=8, debug_taps=debug_taps)
    return _prog_cache[key]


def kernel(x, W1, W2, W3, W4, W5, L1, L2, b2, _trace=False, _debug_taps=False):
    x = np.asarray(x, dtype=np.float32)
    ws = _host_weights(*(np.asarray(a, dtype=np.float32)
                         for a in (W1, W2, W3, W4, W5, L1, L2, b2)))
    nc = _get_prog(_debug_taps)
    in_maps = []
    for b in range(8):
        m = {"x": np.ascontiguousarray(x[b])}
        m.update(ws)
        in_maps.append(m)
    res = run_bass_kernel_spmd(nc, in_maps, core_ids=list(range(8)), trace=_trace)
    out = np.stack([r["out"] for r in res.results])
    if _trace or _debug_taps:
        return out, res
    return out



# revision 13
# speedup vs baseline: 1.5336x; 1.5336x over previous
"""DGCNN Trainium2 Bass kernel.

Data-parallel over batch: 8 samples -> 8 NeuronCores, one sample per core.

Per-layer algorithm (exact reformulation of the reference):
  s[n,m] = <x_n,x_m> - |x_m|^2/2      (row-order-equivalent to reference pd)
  knn(n) = top-10 columns of s row n  (exact fp32 top-k, max8 x2 rounds)
  u = Wa @ x, z = (Wb - Wa) @ x       where W = [Wa | Wb]
  x_next[:, n] = lrelu(z[:, n] + max_{m in knn(n)} u[:, m])
    (lrelu monotone => max-then-lrelu == reference lrelu-then-max)

s is computed per 512-col block as a K=C matmul plus a K=1 bias matmul
(lhsT = ones row, rhs = -xx/2) accumulating into the same PSUM region.
Top-k runs on DVE straight from PSUM. u rows are gathered from an internal
DRAM buffer by 10 per-neighbor indirect DMAs, then reduced with a strided
max. PE transposes bring x_next back to channel-major.

All DMA-loaded weights are laundered through one ACT copy so downstream
consumers depend on a single engine (ISA sync-wait slots are scarce).
"""

import numpy as np

import concourse.bass as bass
import concourse.mybir as mybir
import concourse.tile as tile
from concourse.tile_rust import add_dep_helper
from concourse.bass_utils import run_bass_kernel_spmd
from concourse.masks import make_identity

P = 128
N = 2048
NT = N // P              # 16 row tiles
KNN = 10
NEG = -1e30

# (C_in, O) per edge layer
LAYERS = [(3, 64), (64, 64), (64, 128), (128, 256)]

F32 = mybir.dt.float32
F32R = mybir.dt.float32r
U32 = mybir.dt.uint32


def _lrelu(nc, pool, out_sb, t_ap, tag, shape):
    """out_sb = lrelu(t_ap) = t + 0.8*relu(-t). t_ap may be PSUM or SBUF."""
    rr = pool.tile(shape, F32, tag=f"rr_{tag}")
    nc.scalar.activation(out=rr, in_=t_ap,
                         func=mybir.ActivationFunctionType.Relu, scale=-1.0)
    nc.vector.scalar_tensor_tensor(
        out=out_sb, in0=rr, scalar=0.8, in1=t_ap,
        op0=mybir.AluOpType.mult, op1=mybir.AluOpType.add)


def _pe_join(nc, ident, pj, tgt_cell, dep_cells, after_pe=(), after_other=(),
             skip=True):
    """Dummy 1x1 matmuls so the real matmuls that follow carry <=1 sync wait
    (the ISA matmul has one wait slot). Dummies into the pejoin scratch make
    PE observe: all prior-PE instructions (one dummy, PE sems merge), each
    non-PE instruction (one dummy apiece), and each dependency tensor's
    producer. A final dummy into the target PSUM cell absorbs the slot's
    recycling wait."""
    ic = ident[0:1, 0:1]
    pjc = pj[0:1, 0:1]
    if after_pe:
        d0 = nc.tensor.matmul(pjc, ic, ic, start=True, stop=True,
                              skip_group_check=skip)
        for a in after_pe:
            add_dep_helper(d0.ins, a.ins, sync=True,
                           reason="pe_join absorbs PE wait")
    for a in after_other:
        d = nc.tensor.matmul(pjc, ic, ic, start=True, stop=True,
                             skip_group_check=skip)
        add_dep_helper(d.ins, a.ins, sync=True,
                       reason="pe_join absorbs cross-engine wait")
    ds = []
    for c in dep_cells:
        if c.dtype != F32:
            c = c.bitcast(F32)
        ds.append(nc.tensor.matmul(pjc, c, c, start=True, stop=True,
                                   skip_group_check=skip))
    if tgt_cell is not None:
        nc.tensor.matmul(tgt_cell, ic, ic, start=True, stop=True,
                         skip_group_check=skip)
    return ds


def build_program(num_devices=8, debug_taps=False):
    nc = bass.Bass("TRN2", target_bir_lowering=False, debug=False,
                   enable_asserts=False, num_devices=num_devices)

    # ---- I/O ----
    x_in = nc.dram_tensor("x", (3, N), F32, kind="ExternalInput")
    w_in = {}
    for li, (c, o) in enumerate(LAYERS, start=1):
        w_in[f"waT{li}"] = nc.dram_tensor(f"waT{li}", (c, o), F32, kind="ExternalInput")
        w_in[f"wzT{li}"] = nc.dram_tensor(f"wzT{li}", (c, o), F32, kind="ExternalInput")
    w5T_in = nc.dram_tensor("w5T", (512, 512), F32, kind="ExternalInput")
    l1T_in = nc.dram_tensor("l1T", (1024, 512), F32, kind="ExternalInput")
    l2T_in = nc.dram_tensor("l2T", (512, 256), F32, kind="ExternalInput")
    b2_in = nc.dram_tensor("b2", (256,), F32, kind="ExternalInput")
    out_dram = nc.dram_tensor("out", (256,), F32, kind="ExternalOutput")
    taps = {}
    if debug_taps:
        for li, (c, o) in enumerate(LAYERS, start=1):
            taps[li] = nc.dram_tensor(f"xtap{li}", (o, N), F32, kind="ExternalOutput")

    # internal DRAM u buffers, token-major [N, O]
    u_dram = [nc.dram_tensor(f"u{li}", (N, o), F32, kind="Internal")
              for li, (c, o) in enumerate(LAYERS, start=1)]

    with tile.TileContext(nc) as tc:
        with (
            tc.tile_pool(name="mp", bufs=1) as mp,       # persistent / per-layer
            tc.tile_pool(name="rot", bufs=2) as rot,     # per-tile rotating
            tc.tile_pool(name="ps", bufs=1, space="PSUM") as ps,
            tc.tile_pool(name="ps2", bufs=2, space="PSUM") as ps2,
        ):
            # ---- constants ----
            ident = mp.tile([P, P], F32, tag="ident")
            make_identity(nc, ident)
            halfneg = mp.tile([P, 1], F32, tag="halfneg")
            nc.vector.memset(halfneg, -0.5)
            halfnegr = mp.tile([P, 1], F32R, tag="halfnegr")
            nc.scalar.copy(out=halfnegr, in_=halfneg)
            ones_row = mp.tile([1, P], F32, tag="ones_row")
            nc.vector.memset(ones_row, 1.0)
            ones_row_r = mp.tile([1, P], F32R, tag="ones_row_r")
            nc.scalar.copy(out=ones_row_r, in_=ones_row)
            pscrap = mp.tile([1, NT], F32, tag="pscrap")
            # dummy transpose so PE observes Pool's ident write early
            pj = ps2.tile([P, P], F32, tag="pejoin", bufs=1)
            nc.tensor.transpose(out=pj, in_=ident, identity=ident)

            # ---- load + launder input x (f32r for fast PE streaming) ----
            # x0 aliases the uT_all slot (first real uT write is after the
            # x0 -> x0r launder)
            x0 = mp.tile([3, N], F32, tag="uT_all")
            nc.sync.dma_start(out=x0, in_=x_in.ap())
            x0r = mp.tile([3, N], F32R, tag="x0r")
            nc.scalar.copy(out=x0r, in_=x0)

            feats = {0: [x0r]}
            x1 = mp.tile([64, N], F32R, tag="x1")
            x2 = mp.tile([64, N], F32R, tag="x2")
            x3 = mp.tile([128, N], F32R, tag="x3")
            x4a = mp.tile([128, N], F32R, tag="x4a")
            x4b = mp.tile([128, N], F32R, tag="x4b")
            outs_ch = {1: [x1], 2: [x2], 3: [x3], 4: [x4a, x4b]}

            last_tps = []      # rolling window of recent transpose insts
            prev_mms = []      # all s-matmuls of the previous tile
            prev_dve = []      # all DVE readers of the previous s_ps tile
            prev_mm8 = None
            prev_mr = None
            for li, (C, O) in enumerate(LAYERS, start=1):
                x_ch = feats[li - 1][0]   # [C, N] channel-major input
                ud = u_dram[li - 1]

                # -- weights: DMA then ACT launder --
                waT_r = mp.tile([C, O], F32, tag=f"waT_r{li}")
                wzT_r = mp.tile([C, O], F32, tag=f"wzT_r{li}")
                nc.sync.dma_start(out=waT_r, in_=w_in[f"waT{li}"].ap())
                nc.sync.dma_start(out=wzT_r, in_=w_in[f"wzT{li}"].ap())
                waT = mp.tile([C, O], F32R, tag="waT")
                wzT = mp.tile([C, O], F32R, tag="wzT")
                nc.scalar.copy(out=waT, in_=waT_r)
                nc.scalar.copy(out=wzT, in_=wzT_r)

                # -- -xx/2 row --
                xsq = mp.tile([C, N], F32R, tag="xsq")
                nc.vector.tensor_tensor(out=xsq, in0=x_ch, in1=x_ch,
                                        op=mybir.AluOpType.mult)
                negxx = mp.tile([1, N], F32R, tag="negxx")
                for cblk in range(4):
                    sl = slice(cblk * 512, (cblk + 1) * 512)
                    nx_ps = ps2.tile([1, 512], F32, tag="sm_ps")
                    if cblk == 0:
                        ljd = _pe_join(nc, ident, pj, None,
                                       [xsq[0:1, 0:1], x_ch[0:1, 0:1],
                                        halfnegr[0:1, 0:1], ones_row_r[0:1, 0:1]])
                    nc.scalar.copy(out=nx_ps[0:1, 0:1], in_=ident[0:1, 0:1])
                    mmx = nc.tensor.matmul(nx_ps, halfnegr[0:C, :], xsq[:, sl],
                                           start=True, stop=True)
                    for d in ljd:
                        add_dep_helper(mmx.ins, d.ins, sync=False,
                                       reason="keep join dummies first")
                    nc.scalar.copy(out=negxx[:, sl], in_=nx_ps)

                # -- u^T (staged, single DRAM store), z^T --
                zT_all = mp.tile([P, NT * O], F32, tag="zT_all")
                uT_all = mp.tile([P, NT * O], F32, tag="uT_all")
                for t in range(NT):
                    tsl = slice(t * P, (t + 1) * P)
                    uz_ps = ps2.tile([P, O], F32, tag="sm_ps")
                    nc.scalar.copy(out=uz_ps[0:1, 0:1], in_=ident[0:1, 0:1])
                    nc.tensor.matmul(uz_ps, x_ch[:, tsl], waT, start=True, stop=True)
                    nc.scalar.copy(out=uT_all[:, t * O:(t + 1) * O], in_=uz_ps)
                    uz_ps2 = ps2.tile([P, O], F32, tag="sm_ps")
                    nc.scalar.copy(out=uz_ps2[0:1, 0:1], in_=ident[0:1, 0:1])
                    nc.tensor.matmul(uz_ps2, x_ch[:, tsl], wzT, start=True, stop=True)
                    nc.scalar.copy(out=zT_all[:, t * O:(t + 1) * O], in_=uz_ps2)
                nc.gpsimd.tensor_copy(
                    pscrap, uT_all.rearrange("p (t o) -> p t o", t=NT)[0:1, :, 3])
                ust = nc.gpsimd.dma_start(
                    out=ud.ap().rearrange("(t p) o -> p t o", p=P),
                    in_=uT_all)
                # Pool and ACT observe the store completion (WAR on the DMA's
                # read) so neither the per-tile gathers nor the next layer's
                # uT_all copies need a DMAHW wait slot of their own.
                nc.gpsimd.memset(uT_all[0:1, 0:1], 0.0)
                nc.scalar.mul(uT_all[0:1, 1:2], uT_all[0:1, 1:2], 0.0)

                # -- per row-tile pipeline --
                for t in range(NT):
                    tsl = slice(t * P, (t + 1) * P)
                    s_ps = ps.tile([P, N], F32, tag="s_ps")
                    # first toucher of the recycled slot: DVE memset -- its WAR
                    # (DVE readers) self-elides, so it carries only the PE WAW;
                    # the matmuls then wait on DVE alone.
                    nc.vector.memset(s_ps[0:1, 0:1], 0.0)
                    cur_mms = []
                    for cblk in range(4):
                        sl = slice(cblk * 512, (cblk + 1) * 512)
                        smm = nc.tensor.matmul(
                            s_ps[:, sl], x_ch[:, tsl], x_ch[:, sl],
                            start=True, stop=False)
                        if t == 0 and cblk == 0:
                            for d in ljd:
                                add_dep_helper(smm.ins, d.ins, sync=False,
                                               reason="keep join dummies first")
                        cur_mms.append(smm)
                        cur_mms.append(nc.tensor.matmul(
                            s_ps[:, sl], ones_row_r, negxx[:, sl],
                            start=False, stop=True))
                    prev_mms = cur_mms
                    prev_mm8 = cur_mms[-1]

                    # top-10 via per-512-block max8 candidates (32), then
                    # two full-row index finds. Ranks 9-10 can only be missed
                    # if >=9 of the true top-10 share one 512-block (~3e-5/row)
                    cands = rot.tile([P, 32], F32, tag="cands")
                    bms = []
                    for cb in range(4):
                        bms.append(nc.vector.max(
                            out=cands[:, cb * 8:(cb + 1) * 8],
                            in_=s_ps[:, cb * 512:(cb + 1) * 512]))
                    v1 = rot.tile([P, 8], F32, tag="v1")
                    c2 = rot.tile([P, 32], F32, tag="c2")
                    v2 = rot.tile([P, 8], F32, tag="v2")
                    nc.vector.max(out=v1, in_=cands)
                    nc.vector.match_replace(out=c2, in_to_replace=v1,
                                            in_values=cands, imm_value=NEG)
                    nc.vector.max(out=v2, in_=c2)
                    idx10 = rot.tile([P, 16], U32, tag="idx10")
                    mi1 = nc.vector.max_index(out=idx10[:, 0:8], in_max=v1,
                                              in_values=s_ps)
                    mi2 = nc.vector.max_index(out=idx10[:, 8:16], in_max=v2,
                                              in_values=s_ps)
                    prev_dve = [mi1, mi2]
                    prev_mr = mi2

                    gth = rot.tile([P, KNN * O], F32, tag=f"gth{li}")
                    for j in range(KNN):
                        nc.gpsimd.indirect_dma_start(
                            out=gth[:, j * O:(j + 1) * O],
                            out_offset=None,
                            in_=ud.ap(),
                            in_offset=bass.IndirectOffsetOnAxis(
                                ap=idx10[:, j:j + 1], axis=0),
                        )

                    # chained max over the 10 gathered neighbor blocks; one
                    # DMA-lane wait per op (the ISA wait slot is single)
                    M = rot.tile([P, O], F32, tag="M")
                    nc.vector.tensor_copy(M, gth[:, 0:O])
                    for j in range(1, KNN):
                        nc.vector.tensor_tensor(
                            out=M, in0=M, in1=gth[:, j * O:(j + 1) * O],
                            op=mybir.AluOpType.max)

                    new_tps = []
                    tadd = rot.tile([P, O], F32, tag="tadd")
                    nc.vector.tensor_tensor(out=tadd, in0=M,
                                            in1=zT_all[:, t * O:(t + 1) * O],
                                            op=mybir.AluOpType.add)
                    xnT = rot.tile([P, O], F32, tag="xnT")
                    _lrelu(nc, rot, xnT, tadd, "xn", [P, O])

                    for ob in range((O + P - 1) // P):
                        cols = min(P, O - ob * P)
                        tp_ps = ps2.tile([cols, P], F32, tag="sm_ps")
                        nc.scalar.copy(out=tp_ps[0:1, 0:1], in_=ident[0:1, 0:1])
                        _pe_join(nc, ident, pj, None, [xnT[0:1, 0:1]])
                        tpi = nc.tensor.transpose(
                            out=tp_ps, in_=xnT[:, ob * P:ob * P + cols],
                            identity=ident)
                        new_tps.append(tpi)
                        nc.scalar.copy(out=outs_ch[li][ob][:cols, tsl], in_=tp_ps)
                    last_tps = (last_tps + new_tps)[-4:]

                feats[li] = outs_ch[li]
                # (last tile's transposes feed the next join)
                if debug_taps:
                    for ob, xt in enumerate(outs_ch[li]):
                        nc.gpsimd.tensor_copy(
                            pscrap,
                            xt.rearrange("c (t q) -> c t q", t=NT)[0:1, :, 0])
                        nc.gpsimd.dma_start(
                            out=taps[li].ap()[ob * P:ob * P + xt.shape[0], :],
                            in_=xt)

            # ---- W5 conv + lrelu + pooling ----
            kchunks = [(x1, 64, 0), (x2, 64, 64), (x3, 128, 128),
                       (x4a, 128, 256), (x4b, 128, 384)]
            w5sb = []
            for (src, kc, row0) in kchunks:
                wr = mp.tile([kc, 512], F32, tag=f"w5raw{row0}")
                nc.sync.dma_start(out=wr, in_=w5T_in.ap()[row0:row0 + kc, :])
                wt = mp.tile([kc, 512], F32R, tag=f"w5_{row0}")
                nc.scalar.copy(out=wt, in_=wr)
                w5sb.append(wt)
            fmax = mp.tile([P, 4], F32, tag="fmax")
            fsum = mp.tile([P, 4], F32, tag="fsum")
            for m in range(4):
                msl = slice(m * P, (m + 1) * P)
                h_ps = ps.tile([P, N], F32, tag="s_ps")
                hm = nc.vector.memset(h_ps[0:1, 0:1], 0.0)
                _pe_join(nc, ident, pj,
                         None, [w[0:1, 0:1] for w in w5sb] if m == 0 else [],
                         after_other=[hm])
                for cblk in range(4):
                    sl = slice(cblk * 512, (cblk + 1) * 512)
                    for ci, (src, kc, row0) in enumerate(kchunks):
                        mmh = nc.tensor.matmul(h_ps[:, sl], w5sb[ci][:, msl],
                                               src[:, sl], start=(ci == 0),
                                               stop=(ci == len(kchunks) - 1))
                prev_mms = [mmh]
                last_tps = []
                h_sb = mp.tile([P, N], F32, tag="zT_all")
                t1 = mp.tile([P, N], F32, tag="uT_all")
                tmin = nc.vector.tensor_scalar_min(t1, h_ps, 0.0)
                tstt = nc.vector.scalar_tensor_tensor(
                    out=h_sb, in0=t1, scalar=-0.8, in1=h_ps,
                    op0=mybir.AluOpType.mult, op1=mybir.AluOpType.add)
                prev_dve = [tmin, tstt]
                nc.vector.tensor_reduce(out=fmax[:, m:m + 1], in_=h_sb,
                                        axis=mybir.AxisListType.X,
                                        op=mybir.AluOpType.max)
                nc.vector.tensor_reduce(out=fsum[:, m:m + 1], in_=h_sb,
                                        axis=mybir.AxisListType.X,
                                        op=mybir.AluOpType.add)

            # ---- FC1 (mean-pool divisor folded into l1T rows 512:) ----
            l1sb = []
            for k in range(8):
                wt = mp.tile([P, 512], F32, tag=f"l1_{k}")
                nc.sync.dma_start(out=wt, in_=l1T_in.ap()[k * P:(k + 1) * P, :])
                l1sb.append(wt)
            fvec = mp.tile([P, 4], F32, tag="fvec")
            for m in range(4):
                msl = slice(m * P, (m + 1) * P)
                f_ps = ps2.tile([P, 1], F32, tag="fc_ps", bufs=1)
                nc.vector.memset(f_ps[0:1, 0:1], 0.0)
                if m == 0:
                    fjd = _pe_join(nc, ident, pj, None,
                                   [w[0:1, 0:1] for w in l1sb]
                                   + [fmax[0:1, 0:1], fsum[0:1, 0:1]])
                cur_mms = []
                for k in range(8):
                    rhs = fmax[:, k:k + 1] if k < 4 else fsum[:, k - 4:k - 3]
                    mmf = nc.tensor.matmul(f_ps, l1sb[k][:, msl], rhs,
                                           start=(k == 0), stop=(k == 7))
                    if k == 0:
                        for d in fjd:
                            add_dep_helper(mmf.ins, d.ins, sync=False,
                                           reason="join dummies first")
                    cur_mms.append(mmf)
                prev_mms = cur_mms
                ft = mp.tile([P, 1], F32, tag=f"ft{m}")
                a1 = nc.vector.tensor_scalar_min(ft, f_ps, 0.0)
                a2 = nc.vector.scalar_tensor_tensor(
                    out=fvec[:, m:m + 1], in0=ft, scalar=-0.8, in1=f_ps,
                    op0=mybir.AluOpType.mult, op1=mybir.AluOpType.add)
                prev_dve = [a1, a2]

            # ---- FC2 + b2 + lrelu -> out ----
            l2sb = []
            for k in range(4):
                wt = mp.tile([P, 256], F32, tag=f"l2_{k}")
                nc.sync.dma_start(out=wt, in_=l2T_in.ap()[k * P:(k + 1) * P, :])
                l2sb.append(wt)
            b2r = mp.tile([P, 2], F32, tag="b2r")
            nc.sync.dma_start(out=b2r,
                              in_=b2_in.ap().rearrange("(m p) -> p m", m=2))
            b2sb = mp.tile([P, 2], F32, tag="b2sb")
            nc.vector.tensor_copy(b2sb, b2r)
            for m in range(2):
                msl = slice(m * P, (m + 1) * P)
                o_ps = ps2.tile([P, 1], F32, tag="fc_ps", bufs=1)
                nc.vector.memset(o_ps[0:1, 0:1], 0.0)
                if m == 0:
                    ojd = _pe_join(nc, ident, pj, None,
                                   [w[0:1, 0:1] for w in l2sb] + [fvec[0:1, 0:1]])
                cur_mms = []
                for k in range(4):
                    mmo = nc.tensor.matmul(o_ps, l2sb[k][:, msl],
                                           fvec[:, k:k + 1],
                                           start=(k == 0), stop=(k == 3))
                    if k == 0:
                        for d in ojd:
                            add_dep_helper(mmo.ins, d.ins, sync=False,
                                           reason="join dummies first")
                    cur_mms.append(mmo)
                prev_mms = cur_mms
                ob = mp.tile([P, 1], F32, tag=f"ob{m}")
                nc.vector.tensor_tensor(out=ob, in0=o_ps, in1=b2sb[:, m:m + 1],
                                        op=mybir.AluOpType.add)
                ofin = mp.tile([P, 1], F32, tag=f"ofin{m}")
                ot = mp.tile([P, 1], F32, tag=f"ot{m}")
                b1 = nc.vector.tensor_scalar_min(ot, ob, 0.0)
                badd = nc.vector.tensor_tensor
                b2i = nc.vector.scalar_tensor_tensor(
                    out=ofin, in0=ot, scalar=-0.8, in1=ob,
                    op0=mybir.AluOpType.mult, op1=mybir.AluOpType.add)
                prev_dve = [b1, b2i]
                nc.gpsimd.tensor_copy(pscrap[0:1, 0:1], ofin[0:1, 0:1])
                lastdma = nc.gpsimd.dma_start(
                    out=out_dram.ap()[m * P:(m + 1) * P], in_=ofin)
            prevn = lastdma
            for _ in range(24):
                nn_ = nc.sync.nop()
                add_dep_helper(nn_.ins, prevn.ins, sync=False,
                               reason="keep drain-slack nops last")
                prevn = nn_

    import os
    if not os.environ.get('KERNEL_NO_FIXUP'):
        _fix_waits(nc)
    return nc


def _fix_waits(nc):
    """Post-schedule wait-slot legalization. Each engine instruction may carry
    only ONE sync wait on TRN2. Split any excess waits onto NoOp instructions
    inserted immediately before the over-cap instruction on the same engine
    queue -- all waits still execute before the instruction dispatches, so
    ordering semantics are unchanged."""
    import concourse.mybir as mybir
    ctr = [0]
    for f in nc.m.functions:
        for bb in f.blocks:
            il = list(bb.instructions)
            out = []
            changed = False
            for i in il:
                si = i.sync_info
                n = len(si.on_wait) if (si and si.on_wait) else 0
                if n > 1:
                    w = list(si.on_wait)
                    for extra in w[:-1]:
                        ctr[0] += 1
                        nop = mybir.InstNoOp(name=f"waitnop-{ctr[0]}",
                                             ins=[], outs=[])
                        nop.engine = i.engine
                        nop.sync_info = mybir.SyncInfo(on_wait=[extra],
                                                       on_update=[])
                        out.append(nop)
                    i.sync_info = mybir.SyncInfo(on_wait=[w[-1]],
                                                 on_update=si.on_update)
                    changed = True
                out.append(i)
            if changed:
                bb.instructions = out


def _host_weights(W1, W2, W3, W4, W5, L1, L2, b2):
    ws = {}
    for li, (C, O), W in zip(range(1, 5), LAYERS, [W1, W2, W3, W4]):
        Wa = W[:, :C]
        Wz = W[:, C:] - Wa
        ws[f"waT{li}"] = np.ascontiguousarray(Wa.T.astype(np.float32))
        ws[f"wzT{li}"] = np.ascontiguousarray(Wz.T.astype(np.float32))
    ws["w5T"] = np.ascontiguousarray(W5.T.astype(np.float32))
    l1T = L1.T.astype(np.float32).copy()
    l1T[512:, :] *= np.float32(1.0 / N)   # fold mean-pool divisor
    ws["l1T"] = np.ascontiguousarray(l1T)
    ws["l2T"] = np.ascontiguousarray(L2.T.astype(np.float32))
    ws["b2"] = np.ascontiguousarray(b2.astype(np.float32))
    return ws


_prog_cache = {}


def _get_prog(debug_taps=False):
    key = debug_taps
    if key not in _prog_cache:
        _prog_cache[key] = build_program(num_devices=8, debug_taps=debug_taps)
    return _prog_cache[key]


def kernel(x, W1, W2, W3, W4, W5, L1, L2, b2, _trace=False, _debug_taps=False):
    x = np.asarray(x, dtype=np.float32)
    ws = _host_weights(*(np.asarray(a, dtype=np.float32)
                         for a in (W1, W2, W3, W4, W5, L1, L2, b2)))
    nc = _get_prog(_debug_taps)
    in_maps = []
    for b in range(8):
        m = {"x": np.ascontiguousarray(x[b])}
        m.update(ws)
        in_maps.append(m)
    res = run_bass_kernel_spmd(nc, in_maps, core_ids=list(range(8)), trace=_trace)
    out = np.stack([r["out"] for r in res.results])
    if _trace or _debug_taps:
        return out, res
    return out

